# revision 15
# baseline (speedup 1.0000x reference)
"""Trainium2 Bass kernel for batched multi-head attention (B=8, N=M=C=1024,
H=16, D=64), data-parallel across 8 NeuronCores (one batch element per core).

Per-core dataflow (bf16 matmul inputs, f32 PSUM accumulate), organized as a
single software pipeline so the PE never sees a dependency stall (the PE
queue is strictly in-order, and idle gaps also drop the HAM clock gate to
half speed):

  Prologue: gpsimd SWDGE cast-loads (f32/int32 -> bf16 straight into SBUF)
  + PE transpose-mode matmuls stage q/k/v/target_mask transposed
  (contraction dims on partitions); target_mask lands in an
  [p, nb, mc, n'] layout so the mask multiplies are single-run
  contiguous (DVE 2x mode).

  Steps s = (nb, j) over query-block nb (outer) and head pair j:
    - QK^T for step s as K=64 matmuls with the two heads emitted
      alternately on disjoint PE row groups (concurrent); exp on the
      scalar engine in [128,1024] batches straight from PSUM with the
      1/sqrt(D) scale folded in; target-mask multiply on DVE,
    - projections for step s+1 (dense PE work that buys the scalar
      engine time: q/k head projections, v projection every other pair
      of nb0; key mask folded into vh scaling plus a trailing
      "key-indicator" column so the AV matmul also produces the softmax
      denominator),
    - AV for step s with 65-column lhsT -> numerator + denominator in
      one accumulation chain, then normalization (denominator row
      broadcast by a K=1 f32r ones-matmul, fast reciprocal, multiply),
    - one o-projection accumulation chain of query-block 0 per nb1 step
      (bo folded in as a K=1 ones matmul); the remaining chains drain in
      the epilogue.
"""
import sys

sys.path.insert(0, "/opt/trn_rl_repo")

import numpy as np

import concourse.bass as bass  # noqa: F401
import concourse.mybir as mybir
import concourse.bacc as bacc
import concourse.tile as tile
from concourse import bass_utils

B = 8
N = 1024   # queries
M = 1024   # keys
C = 1024   # model dim
H = 16
D = 64
NP = 8     # head pairs
P = 128
NB = 2     # n blocks of 512
SCALE = D ** -0.5

F32 = mybir.dt.float32
F32R = mybir.dt.float32r
BF16 = mybir.dt.bfloat16
I32 = mybir.dt.int32
MUL = mybir.AluOpType.mult
EXP = mybir.ActivationFunctionType.Exp

_NC_CACHE = {}


def build_nc():
    nc = bacc.Bacc("TRN2", target_bir_lowering=False, debug=False, num_devices=1)

    q_d = nc.dram_tensor("q", [N, C], F32, kind="ExternalInput").ap()
    k_d = nc.dram_tensor("k", [M, C], F32, kind="ExternalInput").ap()
    v_d = nc.dram_tensor("v", [M, C], F32, kind="ExternalInput").ap()
    mask_d = nc.dram_tensor("mask", [M], I32, kind="ExternalInput").ap()
    tm_d = nc.dram_tensor("target_mask", [N, M], I32, kind="ExternalInput").ap()
    wq_d = nc.dram_tensor("Wq", [C, C], F32, kind="ExternalInput").ap()
    wk_d = nc.dram_tensor("Wk", [C, C], F32, kind="ExternalInput").ap()
    wv_d = nc.dram_tensor("Wv", [C, C], F32, kind="ExternalInput").ap()
    wo_d = nc.dram_tensor("Wo", [C, C], F32, kind="ExternalInput").ap()
    bo_d = nc.dram_tensor("bo", [C], F32, kind="ExternalInput").ap()
    out_d = nc.dram_tensor("out", [N, C], F32, kind="ExternalOutput").ap()

    with tile.TileContext(nc) as tc:
        _body(tc, nc, q_d, k_d, v_d, mask_d, tm_d, wq_d, wk_d, wv_d, wo_d,
              bo_d, out_d)
    nc.compile()
    return nc


def _body(tc, nc, q_d, k_d, v_d, mask_d, tm_d, wq_d, wk_d, wv_d, wo_d,
          bo_d, out_d):
    from contextlib import ExitStack
    from concourse.masks import make_identity

    ctx = ExitStack()
    with ctx:
        persist = ctx.enter_context(tc.tile_pool(name="persist", bufs=1))
        lpool = ctx.enter_context(tc.tile_pool(name="lpool", bufs=3))
        wpool = ctx.enter_context(tc.tile_pool(name="wpool", bufs=2))
        wvpool = ctx.enter_context(tc.tile_pool(name="wvpool", bufs=1))
        wcpool = ctx.enter_context(tc.tile_pool(name="wcpool", bufs=2))
        ppool = ctx.enter_context(tc.tile_pool(name="ppool", bufs=2))
        pmpool = ctx.enter_context(tc.tile_pool(name="pmpool", bufs=4))
        npool = ctx.enter_context(tc.tile_pool(name="npool", bufs=3))
        opool = ctx.enter_context(tc.tile_pool(name="opool", bufs=2))
        spsum = ctx.enter_context(tc.tile_pool(name="spsum", bufs=2,
                                               space="PSUM"))
        avpsum = ctx.enter_context(tc.tile_pool(name="avpsum", bufs=2,
                                                space="PSUM"))
        pjpsum = ctx.enter_context(tc.tile_pool(name="pjpsum", bufs=2,
                                                space="PSUM"))

        # ---- persistent tiles ----
        tmT = persist.tile([P, NB, 8, 512], BF16)  # [p, nb, mc, n']
        qbT = persist.tile([P, 8, N], BF16)   # [p, cc, n] = q[n, cc*128+p]
        kbT = persist.tile([P, 8, M], BF16)
        vbT = persist.tile([P, 8, M], BF16)
        qhT = persist.tile([P, NP, N], BF16)  # [p, j, n] = qh[n, j*128+p]
        khT = persist.tile([P, NP, M], BF16)
        vha = persist.tile([P, NP, 8, 130], BF16)
        wob = persist.tile([P, NP, C], BF16)  # [p, j, c2] = Wo[j*128+p, c2]
        xn = persist.tile([P, NB, NP, 512], BF16)

        ident = persist.tile([P, P], BF16)
        make_identity(nc, ident[:])

        mi = persist.tile([P, 8], I32)
        nc.sync.dma_start(out=mi[:], in_=mask_d.rearrange("(mc p) -> p mc", p=P))
        keyf = persist.tile([P, 8], F32)
        nc.vector.tensor_copy(keyf[:], mi[:])
        keyb = persist.tile([P, 8], BF16)
        nc.vector.tensor_copy(keyb[:], keyf[:])

        bo_f = persist.tile([1, C], F32)
        nc.sync.dma_start(out=bo_f[:], in_=bo_d.rearrange("(a c) -> a c", a=1))
        bob = persist.tile([1, C], BF16)
        nc.vector.tensor_copy(bob[:], bo_f[:])

        ones_f = persist.tile([1, D], F32)
        nc.vector.memset(ones_f[:], 1.0)
        onesr = persist.tile([1, D], F32R)
        nc.vector.tensor_copy(onesr[:], ones_f[:])
        onesb = persist.tile([1, P], BF16)
        nc.vector.memset(onesb[:], 1.0)

        # ---- prologue: stage q/k/v/tm on-chip, transposed ----
        def stage_matrix(src_d, write_out):
            src_re = src_d.rearrange("(rc p) c -> p rc c", p=P)
            for rc in range(8):
                for cg in range(2):
                    xb = lpool.tile([P, 512], BF16, tag="ldb")
                    nc.gpsimd.dma_start(
                        out=xb[:], in_=src_re[:, rc, cg * 512:(cg + 1) * 512])
                    tp = pjpsum.tile([P, 4, P], BF16, tag="pj")
                    for ci in range(4):
                        nc.tensor.transpose(tp[:, ci, :],
                                            xb[:, ci * P:(ci + 1) * P],
                                            ident[:])
                    write_out(rc, cg, tp)

        def std_write(dstT):
            def w(rc, cg, tp):
                nc.vector.tensor_copy(
                    dstT[:, cg * 4:(cg + 1) * 4, rc * P:(rc + 1) * P], tp[:])
            return w

        def tm_write(rc, cg, tp):
            # tp[p, ci, n'] = tm[rc*128+n', (cg*4+ci)*128+p]
            nc.vector.tensor_copy(
                tmT[:, rc // 4, cg * 4:(cg + 1) * 4,
                    (rc % 4) * P:(rc % 4 + 1) * P], tp[:])

        stage_matrix(q_d, std_write(qbT))
        stage_matrix(k_d, std_write(kbT))
        stage_matrix(v_d, std_write(vbT))
        stage_matrix(tm_d, tm_write)

        wq_re = wq_d.rearrange("(cc p) c2 -> p cc c2", p=P)
        wk_re = wk_d.rearrange("(cc p) c2 -> p cc c2", p=P)
        wv_re = wv_d.rearrange("(cc p) c2 -> p cc c2", p=P)
        wo_re = wo_d.rearrange("(j p) c2 -> p j c2", p=P)

        # ---------------- pipeline step pieces ----------------
        def proj_pieces(nb, j):
            """Projection work for step (nb, j) as a list of 4 PE-dense
            closures, interleaved between AV chunk pairs by the caller."""
            cs = slice(j * P, (j + 1) * P)
            ns = slice(nb * 512, (nb + 1) * 512)

            def qchain():
                wq_f = wpool.tile([P, 8, P], F32, tag="wf")
                nc.sync.dma_start(out=wq_f[:], in_=wq_re[:, :, cs])
                wqb = wcpool.tile([P, 8, P], BF16, tag="wqb")
                nc.vector.tensor_copy(wqb[:], wq_f[:])
                pq = pjpsum.tile([P, 512], F32, tag="pj")
                for cc in range(8):
                    nc.tensor.matmul(pq[:], wqb[:, cc, :], qbT[:, cc, ns],
                                     start=(cc == 0), stop=(cc == 7))
                nc.scalar.copy(qhT[:, j, ns], pq[:])

            def kchain():
                # keys span all M regardless of the query block: project
                # the FULL khT row for pair j during its first (nb0) step
                wk_f = wpool.tile([P, 8, P], F32, tag="wf")
                nc.sync.dma_start(out=wk_f[:], in_=wk_re[:, :, cs])
                wkb = wcpool.tile([P, 8, P], BF16, tag="wkb")
                nc.vector.tensor_copy(wkb[:], wk_f[:])
                for mb in range(NB):
                    ms = slice(mb * 512, (mb + 1) * 512)
                    pk = pjpsum.tile([P, 512], F32, tag="pj")
                    for cc in range(8):
                        nc.tensor.matmul(pk[:], wkb[:, cc, :], kbT[:, cc, ms],
                                         start=(cc == 0), stop=(cc == 7))
                    nc.scalar.copy(khT[:, j, ms], pk[:])
                wo_f = wpool.tile([P, 8, P], F32, tag="wf")
                nc.sync.dma_start(
                    out=wo_f[:].rearrange("p a b -> p (a b)"),
                    in_=wo_re[:, j, :])
                nc.scalar.copy(wob[:, j, :],
                               wo_f[:].rearrange("p a b -> p (a b)"))

            if nb == 1:
                return [qchain, None, None, None]
            if j % 2 == 1:
                return [qchain, kchain, None, None]

            # v projection -> vha for pairs (j, j+1), once (during nb0);
            # Wv loaded in two 4KB halves through the shared weight slots.
            wvb = wvpool.tile([P, 8, 256], BF16, tag="wvb")

            def vload():
                for h in range(2):
                    wv_f = wpool.tile([P, 8, P], F32, tag="wf")
                    nc.sync.dma_start(
                        out=wv_f[:],
                        in_=wv_re[:, :, (j + h) * P:(j + h + 1) * P])
                    nc.vector.tensor_copy(wvb[:, :, h * P:(h + 1) * P],
                                          wv_f[:])

            def vhalf(mc0):
                def run():
                    for mc in range(mc0, mc0 + 4):
                        pv = pjpsum.tile([P, 256], F32, tag="pj")
                        for cc in range(8):
                            nc.tensor.matmul(
                                pv[:], vbT[:, cc, mc * P:(mc + 1) * P],
                                wvb[:, cc, :],
                                start=(cc == 0), stop=(cc == 7))
                        out_sl = vha[:, j:j + 2, mc, :].rearrange(
                            "p j (hx dd) -> p j hx dd", hx=2)[:, :, :, 0:64]
                        in_sl = pv[:].rearrange("p (j hx dd) -> p j hx dd",
                                                j=2, hx=2)
                        nc.vector.tensor_scalar(out_sl, in_sl,
                                                keyf[:, mc:mc + 1], None,
                                                op0=MUL)
                    if mc0 == 4:
                        for jx in (j, j + 1):
                            nc.vector.tensor_copy(vha[:, jx, :, 64], keyb[:])
                            nc.vector.tensor_copy(vha[:, jx, :, 129], keyb[:])
                return run

            def qchain_v():
                vload()
                qchain()

            return [qchain_v, kchain, vhalf(0), vhalf(4)]

        def emit_qk(nb, j):
            """QK^T + exp + mask for step (nb, j); returns pm tiles."""
            ns = slice(nb * 512, (nb + 1) * 512)
            pms = []
            for kcp in range(4):
                sp0 = spsum.tile([P, 2, 512], F32, tag="sp")
                sp1 = spsum.tile([P, 2, 512], F32, tag="sp")
                sps = [sp0, sp1]
                for kcx in range(2):
                    kc = 2 * kcp + kcx
                    msl = slice(kc * P, (kc + 1) * P)
                    for hx in range(2):
                        rows = slice(hx * 64, (hx + 1) * 64)
                        nc.tensor.matmul(sps[hx][:, kcx, :],
                                         khT[rows, j, msl],
                                         qhT[rows, j, ns],
                                         start=True, stop=True)
                for hx in range(2):
                    pb = ppool.tile([P, 2, 512], BF16, tag="pb")
                    nc.scalar.activation(pb[:], sps[hx][:], EXP, scale=SCALE)
                    pm = pmpool.tile([P, 2, 512], BF16, tag="pm")
                    nc.vector.tensor_tensor(
                        pm[:].rearrange("p a b -> p (a b)"),
                        pb[:].rearrange("p a b -> p (a b)"),
                        tmT[:, nb, 2 * kcp:2 * kcp + 2, :].rearrange(
                            "p a b -> p (a b)"), MUL)
                    pms.append(pm)
            return pms

        def emit_av(nb, j, pms, pieces):
            """AV + normalize for step (nb, j), with the next step's
            projection pieces interleaved between AV chunk pairs so the
            in-order PE queue always has ready work while the scalar
            engine computes exp."""
            av0 = avpsum.tile([65, 512], F32, tag="av")
            av1 = avpsum.tile([65, 512], F32, tag="av")
            for kcp in range(4):
                if pieces[kcp] is not None:
                    pieces[kcp]()
                for kcx in range(2):
                    kc = 2 * kcp + kcx
                    nc.tensor.matmul(av0[:], vha[:, j, kc, 0:65],
                                     pms[2 * kcp][:, kcx, :],
                                     start=(kc == 0), stop=(kc == 7))
                for kcx in range(2):
                    kc = 2 * kcp + kcx
                    nc.tensor.matmul(av1[:], vha[:, j, kc, 65:130],
                                     pms[2 * kcp + 1][:, kcx, :],
                                     start=(kc == 0), stop=(kc == 7))
            for hx, av in ((0, av0), (1, av1)):
                dd = npool.tile([1, 512], F32R, tag="rd")
                nc.vector.tensor_copy(dd[:], av[64:65, :])
                bc = spsum.tile([64, 512], F32, tag="sp")
                nc.tensor.matmul(bc[:], onesr[0:1, :], dd[:],
                                 start=True, stop=True)
                rc = npool.tile([64, 512], F32, tag="rc")
                nc.vector.reciprocal_approx_fast(rc[:], bc[:])
                rows = slice(hx * 64, (hx + 1) * 64)
                nc.vector.tensor_tensor(xn[rows, nb, j, :],
                                        av[0:64, :], rc[:], MUL)

        def emit_ochain(nb, chain):
            """One o-projection chain (chain = nch*2 + c2h) for block nb."""
            nch, c2h = chain // 2, chain % 2
            nsl = slice(nch * P, (nch + 1) * P)
            c2s = slice(c2h * 512, (c2h + 1) * 512)
            po = pjpsum.tile([P, 512], F32, tag="pj")
            nc.tensor.matmul(po[:], onesb[0:1, :], bob[0:1, c2s],
                             start=True, stop=False)
            for j in range(NP):
                nc.tensor.matmul(po[:], xn[:, nb, j, nsl], wob[:, j, c2s],
                                 start=False, stop=(j == NP - 1))
            ot = opool.tile([P, 512], F32, tag="ot")
            if c2h == 0:
                nc.scalar.copy(ot[:], po[:])
            else:
                nc.vector.tensor_copy(ot[:], po[:])
            nc.sync.dma_start(
                out=out_d[nb * 512 + nch * P:nb * 512 + (nch + 1) * P, c2s],
                in_=ot[:])

        # ---------------- the pipeline ----------------
        steps = [(nb, j) for nb in range(NB) for j in range(NP)]
        for piece in proj_pieces(*steps[0]):
            if piece is not None:
                piece()
        for s, (nb, j) in enumerate(steps):
            pms = emit_qk(nb, j)
            if s + 1 < len(steps):
                pieces = proj_pieces(*steps[s + 1])
            else:
                pieces = [None] * 4
            if nb == 1:
                # o-projection chain of block 0 fills the slot that k/v
                # projections occupy during nb0 steps
                pieces[1] = (lambda jj: (lambda: emit_ochain(0, jj)))(j)
            emit_av(nb, j, pms, pieces)
        for chain in range(8):
            emit_ochain(1, chain)


def _get_nc():
    if "nc" not in _NC_CACHE:
        _NC_CACHE["nc"] = build_nc()
    return _NC_CACHE["nc"]


def _in_maps(q, k, v, mask, target_mask, Wq, Wk, Wv, Wo, bo):
    shared = {
        "Wq": np.ascontiguousarray(np.asarray(Wq, np.float32)),
        "Wk": np.ascontiguousarray(np.asarray(Wk, np.float32)),
        "Wv": np.ascontiguousarray(np.asarray(Wv, np.float32)),
        "Wo": np.ascontiguousarray(np.asarray(Wo, np.float32)),
        "bo": np.ascontiguousarray(np.asarray(bo, np.float32)),
    }
    q = np.ascontiguousarray(np.asarray(q, np.float32))
    k = np.ascontiguousarray(np.asarray(k, np.float32))
    v = np.ascontiguousarray(np.asarray(v, np.float32))
    mask = np.ascontiguousarray(np.asarray(mask, np.int32))
    target_mask = np.ascontiguousarray(np.asarray(target_mask, np.int32))
    in_maps = []
    for b in range(B):
        m = {"q": q[b], "k": k[b], "v": v[b], "mask": mask[b],
             "target_mask": target_mask[b]}
        m.update(shared)
        in_maps.append(m)
    return in_maps


def kernel(q, k, v, mask, target_mask, Wq, Wk, Wv, Wo, bo):
    nc = _get_nc()
    in_maps = _in_maps(q, k, v, mask, target_mask, Wq, Wk, Wv, Wo, bo)
    res = bass_utils.run_bass_kernel_spmd(nc, in_maps, core_ids=list(range(B)))
    out = np.stack([res.results[b]["out"] for b in range(B)], axis=0)
    return out.astype(np.float32)


def run_traced(q, k, v, mask, target_mask, Wq, Wk, Wv, Wo, bo, **trace_kwargs):
    """Like kernel() but with NTFF tracing; returns (out, BassKernelResults)."""
    nc = _get_nc()
    in_maps = _in_maps(q, k, v, mask, target_mask, Wq, Wk, Wv, Wo, bo)
    res = bass_utils.run_bass_kernel_spmd(nc, in_maps, core_ids=list(range(B)),
                                          trace=True, **trace_kwargs)
    out = np.stack([res.results[b]["out"] for b in range(B)], axis=0)
    return out.astype(np.float32), res
